# revision 16
# baseline (speedup 1.0000x reference)
"""Trainium2 Bass kernel for nn_DLGN_VT (deep linearly-gated network w/ value tensor).

Math (per batch row b):
    g_i = sigmoid(30 * x @ W_i.T)            i = 1,2,3    [B, 32] each
    out[b] = sum_{ijk} g1[b,i] g2[b,j] g3[b,k] V[i,j,k]

Distribution: pure data-parallel over the batch axis, 8 NeuronCores,
512 rows per core. W_i and V are tiny and replicated.

Per-core algorithm (host-side layout prep is free):
  - Each DMA-initiating engine (scalar/gpsimd/sync) owns one hardware queue,
    so the critical x payload is split three ways and enqueued in parallel:
    xca [128, 352] bf16 = Wh | xh-left     (scalar queue)
    xcb [128, 256] bf16 = xh-right         (gpsimd queue)
    xcc [128, 608] bf16 = Wl | xl          (sync queue, then cba, cbb)
    (hi/lo bf16 split of the xT shard / Wall^T(W2;W3;W1) for error-compensated
    gating.)  cba = V^T chunks + S3; cbb = S2 + ones.
  - Dummy matmuls warm the PE HAM clock gate during the DMA wait.
  - Error-compensated bf16 gating (Wh.xh + Wl.xh + Wh.xl), column-split so
    each xh half is consumed as soon as its DMA lands: Gps[96n, 512b].
  - ONE sigmoid (scale=30) over all 96 logits -> g1/g2/g3 bf16 [96, 512].
  - A^T[(jk), b] = g2[j,b]*g3[k,b] in 4 pair-blocks, split across DVE + ACT:
    E2 = S2_q.T @ g2t selection matmuls fill PSUM pair tiles; E3 = S3.T @ g3t
    is copied once to SBUF bf16 and duplicated to [128, 1024].
      pair 0: DVE tensor_tensor straight from PSUM (1x rate).
      pairs 1-3: ACT copies the PSUM pair to SBUF bf16 (runs parallel to the
        DVE), then the DVE multiplies bf16 x bf16 in 2x packed mode (~690ns
        vs 1224ns) -- both engines stream concurrently.
  - C^T[i, b] = sum_jk V[i,jk] A^T[jk,b] accumulates over 8 bf16 matmuls with
    host-transposed V chunks as the stationary operand.
  - out[b] = sum_i g1t[i,b] * C^T[i,b]: one VectorE multiply + ones-matmul,
    PSUM->SBUF copy split ACT/DVE, single-packet DMA to DRAM.
"""

import numpy as np
import ml_dtypes

import concourse.bass as bass
import concourse.bacc as bacc
import concourse.mybir as mybir
import concourse.tile as tile
from concourse.alu_op_type import AluOpType
from concourse.bass_utils import run_bass_kernel_spmd

BF16 = ml_dtypes.bfloat16
NCORES = 8
B, D, N = 4096, 128, 32
BL = B // NCORES  # 512 batch rows per core
HB = BL // 2      # column split of the gating matmuls
BETA = 30.0
NQ = 8   # 128-row blocks of the jk=1024 plane
NP = 4   # pairs of blocks

F32 = mybir.dt.float32
DBF = mybir.dt.bfloat16

# xca: packed bf16 [128, 352]: Wh | xh-left
WH0, WH1 = 0, 96
XH0, XH1 = WH1, WH1 + HB
CA1 = XH1
# xcc: packed bf16 [128, 608]: Wl | xl
WL0, WL1 = 0, 96
XL0, XL1 = WL1, WL1 + BL
CC1 = XL1
# cba: bf16 [128, 384]: V^T chunks [128, 256] | S3 at rows 32:64, cols 256:384
VT0, VT1 = 0, 256
S30, S31 = 256, 384
# cbb: bf16 [32, 1025]: S2 selections [32, 1024] | ones [32, 1]
S20, S21 = 0, 1024
ON0 = 1024

N_WARMUP = 5   # dummy matmuls to warm the PE HAM clock gate
OCL = 192      # out-copy columns done on ACT (rest on DVE)


def build_nc():
    # Bacc (not raw Bass): its compile passes split multi-wait sync infos
    # (TRN2 allows at most one sync wait per compute instruction).
    nc = bacc.Bacc(None)
    xca_d = nc.declare_dram_parameter("xca", [128, CA1], DBF, isOutput=False)
    xcb_d = nc.declare_dram_parameter("xcb", [128, HB], DBF, isOutput=False)
    xcc_d = nc.declare_dram_parameter("xcc", [128, CC1], DBF, isOutput=False)
    cba_d = nc.declare_dram_parameter("cba", [128, 384], DBF, isOutput=False)
    cbb_d = nc.declare_dram_parameter("cbb", [32, 1025], DBF, isOutput=False)
    out_d = nc.declare_dram_parameter("out", [1, BL], F32, isOutput=True)

    sig = mybir.ActivationFunctionType.Sigmoid

    with tile.TileContext(nc) as tc:
        with (
            tc.tile_pool(name="const", bufs=1) as cpool,
            tc.tile_pool(name="work", bufs=1) as wpool,
            tc.tile_pool(name="atp", bufs=1) as apool,
            tc.tile_pool(name="psA", bufs=2, space="PSUM") as psA,
            tc.tile_pool(name="psB", bufs=2, space="PSUM") as psB,
            tc.tile_pool(name="psC", bufs=1, space="PSUM") as psC,
            # psA: gps/cps share bank 0, e3ps/dum/ops cycle bank 1 & 0.
            # psB (4 banks): pair0 (DVE) + pair2 (ACT), pair3 reuses pair0's
            # slot after its TT.  psC (2 banks): pair1 (DVE).  Total 8.
        ):
            xca = cpool.tile([128, CA1], DBF)
            xcb = cpool.tile([128, HB], DBF)
            xcc = cpool.tile([128, CC1], DBF)
            cba = cpool.tile([128, 384], DBF)
            cbb = cpool.tile([32, 1025], DBF)
            wz = wpool.tile([128, BL], DBF)

            # Parallel DMA enqueue, one hardware queue per engine; the wz
            # memset runs first on gpsimd so the PE warmups start early.
            nc.scalar.dma_start(xca[:], xca_d[:])
            nc.gpsimd.memset(wz[:], 0.0)
            nc.gpsimd.dma_start(xcb[:], xcb_d[:])
            nc.sync.dma_start(xcc[:], xcc_d[:])
            nc.sync.dma_start(cba[:], cba_d[:])
            nc.sync.dma_start(cbb[:], cbb_d[:])

            wh = xca[:, WH0:WH1]
            xh0 = xca[:, XH0:XH1]
            xh1 = xcb[:]
            wl = xcc[:, WL0:WL1]
            xl = xcc[:, XL0:XL1]
            vts = cba[:, VT0:VT1]            # [128, 8*32] V^T chunks (C lhsT)
            s3 = cba[32:64, S30:S31]         # [32, 128] E3 selection (base 32)
            s2 = cbb[:, S20:S21]             # [32, 8*128] E2 selection blocks
            ones = cbb[:, ON0 : ON0 + 1]     # [32, 1]

            # ---- PE warmup in the gating PSUM bank (overwritten later) ----
            gps = psA.tile([96, BL], F32, tag="ps")
            for _ in range(N_WARMUP):
                nc.tensor.matmul(gps[:, :], wz[:, 0:96], wz[:],
                                 start=True, stop=True)

            # ---- gating: error-compensated bf16 matmul, column-split so
            # each xh half starts as soon as its own DMA lands.  Only the
            # first matmul carries start=True (start clears the whole PSUM
            # bank); the rest rely on per-element has_written semantics (the
            # warmups leave exact zeros).  The full-width Wh.xl pass runs
            # last and carries the stop for every column. ----
            nc.tensor.matmul(gps[:, 0:HB], wh, xh0, start=True, stop=False)
            nc.tensor.matmul(gps[:, HB:BL], wh, xh1, start=False, stop=False)
            nc.tensor.matmul(gps[:, 0:HB], wl, xh0, start=False, stop=False)
            nc.tensor.matmul(gps[:, HB:BL], wl, xh1, start=False, stop=False)
            nc.tensor.matmul(gps[:], wh, xl, start=False, stop=True)

            # ---- one sigmoid for all 96 gates (bf16 is plenty for 2e-2) ----
            g123 = wpool.tile([3 * N, BL], DBF)
            nc.scalar.activation(g123[:], gps[:], sig, scale=BETA)
            g2t = g123[0:32, :]
            g3t = g123[32:64, :]  # base partition 32, matching s3
            g1t = g123[64:96, :]

            # ---- E3 = S3.T @ g3t -> DVE cast to bf16 + one 4x bf16
            # duplicate, giving e3d [128, 2, BL] for the TTs ----
            e3ps = psA.tile([128, BL], F32, tag="ps")
            nc.tensor.matmul(e3ps[:], s3, g3t, start=True, stop=True)
            e3d = wpool.tile([128, 2, BL], DBF)
            nc.vector.tensor_copy(e3d[:, 0, :], e3ps[:])
            nc.vector.tensor_copy(e3d[:, 1, :], e3d[:, 0, :])

            # ---- E2 selection matmuls.  Pairs 0/1 (DVE-direct) first, then
            # pair 2 (ACT path); pair 3's matmuls reuse pair 0's PSUM slot
            # after pair 0's TT has drained it. ----
            e2p0 = psB.tile([128, 2, BL], F32, tag="e2")
            e2p1 = psC.tile([128, 2, BL], F32, tag="p1")
            e2p2 = psB.tile([128, 2, BL], F32, tag="e2")
            for h in range(2):
                nc.tensor.matmul(
                    e2p0[:, h, :], s2[:, 128 * h : 128 * (h + 1)], g2t,
                    start=True, stop=True,
                )
            for h in range(2):
                q = 2 + h
                nc.tensor.matmul(
                    e2p1[:, h, :], s2[:, 128 * q : 128 * (q + 1)], g2t,
                    start=True, stop=True,
                )
            for h in range(2):
                q = 4 + h
                nc.tensor.matmul(
                    e2p2[:, h, :], s2[:, 128 * q : 128 * (q + 1)], g2t,
                    start=True, stop=True,
                )

            # HAM warm-keepers: tiny matmuls into a scratch PSUM tile keep
            # the PE clock gate from re-throttling while the PE waits for
            # pair 0's TT to free the PSUM slot pair 3 needs.  Their scratch
            # tile depends on e3ps being drained, so the scheduler cannot
            # hoist them ahead of the real work.
            dum = psA.tile([96, BL], F32, tag="ps")
            for _ in range(3):
                nc.tensor.matmul(dum[:, 0:128], wz[:, 0:96], wz[:, 0:128],
                                 start=True, stop=True)

            e2p3 = psB.tile([128, 2, BL], F32, tag="e2")
            for h in range(2):
                q = 6 + h
                nc.tensor.matmul(
                    e2p3[:, h, :], s2[:, 128 * q : 128 * (q + 1)], g2t,
                    start=True, stop=True,
                )

            ats = [
                apool.tile([128, 2, BL], DBF, tag=f"at_{p}", name=f"at_{p}")
                for p in range(NP)
            ]
            # pairs 0/1: DVE TT straight from PSUM (fp32 x bf16, 1x rate)
            nc.vector.tensor_tensor(ats[0][:], e2p0[:], e3d[:], AluOpType.mult)
            nc.vector.tensor_tensor(ats[1][:], e2p1[:], e3d[:], AluOpType.mult)
            # pairs 2/3: ACT copies PSUM -> SBUF bf16 (parallel engine), DVE
            # multiplies in 2x packed bf16 mode
            for p, e2ps in ((2, e2p2), (3, e2p3)):
                e2sb = wpool.tile([128, 2, BL], DBF, tag=f"e2sb_{p}",
                                  name=f"e2sb_{p}")
                nc.scalar.copy(e2sb[:], e2ps[:])
                nc.vector.tensor_tensor(ats[p][:], e2sb[:], e3d[:], AluOpType.mult)

            # ---- C accumulation over the 8 blocks (two more warm-keepers
            # cover the wait for the last bf16 pair) ----
            cps = psA.tile([N, BL], F32, tag="ps")
            for q in range(NQ):
                p, h = q // 2, q % 2
                nc.tensor.matmul(
                    cps[:], vts[:, 32 * q : 32 * (q + 1)], ats[p][:, h, :],
                    start=(q == 0), stop=(q == NQ - 1),
                )
                if q == 5:
                    for _ in range(2):
                        nc.tensor.matmul(dum[:, 0:128], wz[:, 0:96],
                                         wz[:, 0:128], start=True, stop=True)

            # ---- out = ones.T @ (g1t .* C^T); one DVE copy out of PSUM
            # (splitting the copy across engines just re-serializes on the
            # shared outs tile) ----
            y = wpool.tile([N, BL], DBF)
            nc.vector.tensor_tensor(y[:], cps[:], g1t[:], AluOpType.mult)
            ops = psA.tile([1, BL], F32, tag="ps")
            nc.tensor.matmul(ops[:], ones, y[:], start=True, stop=True)
            outs = wpool.tile([1, BL], F32)
            nc.vector.tensor_copy(outs[:], ops[:])
            nc.sync.dma_start(out_d[:], outs[:], single_packet=True)

    nc.finalize()
    return nc


def host_prep(x, W1, W2, W3, V):
    """Build per-core input maps (all numpy, fp32 in / packed layouts out)."""
    x = np.asarray(x, dtype=np.float32)
    W1 = np.asarray(W1, dtype=np.float32)
    W2 = np.asarray(W2, dtype=np.float32)
    W3 = np.asarray(W3, dtype=np.float32)
    V = np.asarray(V, dtype=np.float32)

    xT = np.ascontiguousarray(x.T)  # [128, 4096]

    # order: g2 rows first (E2-mm rhs at base partition 0), then g3 (base 32,
    # matching the S3 placement), then g1 (only needed at the very end)
    Wall = np.concatenate([W2, W3, W1], axis=0)  # [96, 128]
    cf = np.ascontiguousarray(Wall.T)  # [128, 96] fp32

    # V^T chunks: VTs[p, 32q + i] = V[0, i, j, k] with jk = 128q + p
    Vr = V.reshape(N, N * N)  # [i, jk]
    VT = np.ascontiguousarray(Vr.T)  # [jk, i]
    VTs = VT.reshape(NQ, 128, N).transpose(1, 0, 2).reshape(128, NQ * N)

    # E2 selection: S2[j', q*128 + p] = 1 iff j' == 4q + p//32
    S2 = np.zeros((N, NQ, 128), dtype=np.float32)
    for q in range(NQ):
        for p in range(128):
            S2[4 * q + p // 32, q, p] = 1.0
    S2pack = S2.reshape(N, NQ * 128)

    # E3 selection: S3[k', p] = 1 iff k' == p % 32
    S3 = np.zeros((N, 128), dtype=np.float32)
    for p in range(128):
        S3[p % 32, p] = 1.0

    cba = np.zeros((128, 384), dtype=BF16)
    cba[:, VT0:VT1] = VTs.astype(BF16)
    cba[32:64, S30:S31] = S3.astype(BF16)

    cbb = np.zeros((32, 1025), dtype=BF16)
    cbb[:, S20:S21] = S2pack.astype(BF16)
    cbb[:, ON0] = np.ones(N, dtype=BF16)

    wh = cf.astype(BF16)
    wl = (cf - wh.astype(np.float32)).astype(BF16)

    xca = np.zeros((128, CA1), dtype=BF16)
    xca[:, WH0:WH1] = wh
    xcc0 = np.zeros((128, CC1), dtype=BF16)
    xcc0[:, WL0:WL1] = wl

    in_maps = []
    for c in range(NCORES):
        ma = xca.copy()
        mc = xcc0.copy()
        xs = xT[:, c * BL : (c + 1) * BL]
        xhc = xs.astype(BF16)
        xlc = (xs - xhc.astype(np.float32)).astype(BF16)
        ma[:, XH0:XH1] = xhc[:, 0:HB]
        mc[:, XL0:XL1] = xlc
        in_maps.append(
            {"xca": ma, "xcb": np.ascontiguousarray(xhc[:, HB:BL]),
             "xcc": mc, "cba": cba, "cbb": cbb}
        )
    return in_maps


_CACHED_NC = None


def _ensure_ntff_hook():
    """The agent image's `antenv` package lacks `axon_hooks`; synthesize it
    and register the boot module's ctypes-based NTFF profile hook so
    run_bass_kernel_spmd(trace=True) can capture neuron-profile output."""
    import sys, types

    try:
        from antenv.axon_hooks import get_axon_ntff_profile_hook  # noqa: F401

        return
    except ImportError:
        pass
    import antenv
    from trn_agent_boot.trn_boot import _ntff_profile_via_ctypes

    mod = types.ModuleType("antenv.axon_hooks")
    mod._hook = _ntff_profile_via_ctypes("/opt/axon/libaxon_pjrt.so")
    mod.get_axon_ntff_profile_hook = lambda: mod._hook
    mod.set_axon_ntff_profile_hook = lambda h: setattr(mod, "_hook", h)
    sys.modules["antenv.axon_hooks"] = mod
    antenv.axon_hooks = mod


def run(inputs, trace=False, **trace_kwargs):
    """Run the kernel on 8 cores. Returns (out [4096] f32, BassKernelResults)."""
    global _CACHED_NC
    if trace:
        _ensure_ntff_hook()
    if _CACHED_NC is None:
        _CACHED_NC = build_nc()
    in_maps = host_prep(
        inputs["x"], inputs["W1"], inputs["W2"], inputs["W3"], inputs["V"]
    )
    res = run_bass_kernel_spmd(
        _CACHED_NC, in_maps, core_ids=list(range(NCORES)), trace=trace, **trace_kwargs
    )
    out = np.concatenate(
        [np.asarray(res.results[c]["out"]).reshape(BL) for c in range(NCORES)]
    ).astype(np.float32)
    return out, res


def kernel(**inputs):
    out, _ = run(inputs, trace=False)
    return out


# revision 19
# speedup vs baseline: 1.0040x; 1.0040x over previous
"""Trainium2 Bass kernel for nn_DLGN_VT (deep linearly-gated network w/ value tensor).

Math (per batch row b):
    g_i = sigmoid(30 * x @ W_i.T)            i = 1,2,3    [B, 32] each
    out[b] = sum_{ijk} g1[b,i] g2[b,j] g3[b,k] V[i,j,k]

Distribution: pure data-parallel over the batch axis, 8 NeuronCores,
512 rows per core. W_i and V are tiny and replicated.

Per-core algorithm (host-side layout prep is free):
  - Each DMA-initiating engine (scalar/gpsimd/sync) owns one hardware queue,
    so the critical x payload is split three ways and enqueued in parallel:
    xca [128, 352] bf16 = Wh | xh-left     (scalar queue)
    xcb [128, 256] bf16 = xh-right         (gpsimd queue)
    xcc [128, 608] bf16 = Wl | xl          (sync queue, then cba, s23c, cbb)
  - Dummy matmuls warm the PE HAM clock gate during the DMA wait.
  - Error-compensated bf16 gating (Wh.xh + Wl.xh + Wh.xl), column-split so
    each xh half is consumed as soon as its DMA lands: Gps[96n, 512b].
  - ONE sigmoid (scale=30) over all 96 logits -> g1/g2/g3 bf16 [96, 512].
  - The A^T[(jk), b] = g2[j,b]*g3[k,b] plane (8 blocks of 128 partitions) is
    built by two engines concurrently:
      blocks 0-5 (DVE): E2 = S2_q.T @ g2t selection matmuls into PSUM,
        E3 = S3.T @ g3t copied once to SBUF, one tensor_tensor per pair
        (the last pair split in half so the C chain drains sooner).
      blocks 6-7 (ACT): L = ln(sigma + 1e-37) on the scalar engine, then
        one matmul per block SUMS L2[j]+L3[k] in PSUM (log-domain product),
        and exp() writes sigma2*sigma3 straight to SBUF bf16.  ln/exp share
        one ACT table set; the single table switch after the sigmoid hides
        behind the DVE stream.  The 1e-37 bias keeps ln() finite where bf16
        sigma underflows to 0, so no NaN can leak through the selection
        matmul's zero-weight lanes.
  - C^T[i, b] = sum_jk V[i,jk] A^T[jk,b] accumulates over 8 bf16 matmuls,
    ordered by expected block arrival.
  - out[b] = sum_i g1t[i,b] * C^T[i,b]: one VectorE multiply + ones-matmul,
    one DVE copy out of PSUM, single-packet DMA to DRAM.
"""

import numpy as np
import ml_dtypes

import concourse.bass as bass
import concourse.bacc as bacc
import concourse.mybir as mybir
import concourse.tile as tile
from concourse.alu_op_type import AluOpType
from concourse.bass_utils import run_bass_kernel_spmd

BF16 = ml_dtypes.bfloat16
NCORES = 8
B, D, N = 4096, 128, 32
BL = B // NCORES  # 512 batch rows per core
HB = BL // 2      # column split of the gating matmuls
BETA = 30.0
NQ = 8   # 128-row blocks of the jk=1024 plane

F32 = mybir.dt.float32
DBF = mybir.dt.bfloat16

# xca: packed bf16 [128, 352]: Wh | xh-left
WH0, WH1 = 0, 96
XH0, XH1 = WH1, WH1 + HB
CA1 = XH1
# xcc: packed bf16 [128, 608]: Wl | xl
WL0, WL1 = 0, 96
XL0, XL1 = WL1, WL1 + BL
CC1 = XL1
# cba: bf16 [128, 384]: V^T chunks [128, 256] | S3 at rows 32:64, cols 256:384
VT0, VT1 = 0, 256
S30, S31 = 256, 384
# cbb: bf16 [32, 769]: S2 selections for blocks 0-5 | ones [32, 1]
S20, S21 = 0, 768
ON0 = 768
# s23c: bf16 [64, 256]: summed-log selections for blocks 6, 7

N_WARMUP = 4  # dummy matmuls to warm the PE HAM clock gate
LNB = 1e-37   # ln() bias: keeps L finite when bf16 sigma underflows to 0


def build_nc():
    # Bacc (not raw Bass): its compile passes split multi-wait sync infos
    # (TRN2 allows at most one sync wait per compute instruction).
    nc = bacc.Bacc(None)
    xca_d = nc.declare_dram_parameter("xca", [128, CA1], DBF, isOutput=False)
    xcb_d = nc.declare_dram_parameter("xcb", [128, HB], DBF, isOutput=False)
    xcc_d = nc.declare_dram_parameter("xcc", [128, CC1], DBF, isOutput=False)
    cba_d = nc.declare_dram_parameter("cba", [128, 384], DBF, isOutput=False)
    s23c_d = nc.declare_dram_parameter("s23c", [64, 256], DBF, isOutput=False)
    cbb_d = nc.declare_dram_parameter("cbb", [32, ON0 + 1], DBF, isOutput=False)
    out_d = nc.declare_dram_parameter("out", [1, BL], F32, isOutput=True)

    sig = mybir.ActivationFunctionType.Sigmoid
    fln = mybir.ActivationFunctionType.Ln
    fexp = mybir.ActivationFunctionType.Exp

    with tile.TileContext(nc) as tc:
        with (
            tc.tile_pool(name="const", bufs=1) as cpool,
            tc.tile_pool(name="work", bufs=1) as wpool,
            tc.tile_pool(name="atp", bufs=1) as apool,
            tc.tile_pool(name="psA", bufs=2, space="PSUM") as psA,
            tc.tile_pool(name="psB", bufs=2, space="PSUM") as psB,
            tc.tile_pool(name="psP", bufs=2, space="PSUM") as psP,
        ):
            # PSUM budget (8 banks): psA 2 (gps/cps share one slot, e3ps/ops
            # the other), psB 4 (pairs 0/1; pair 2 reuses pair 0's slot
            # after its TT), psP 2 (log-sum blocks 6/7).
            xca = cpool.tile([128, CA1], DBF)
            xcb = cpool.tile([128, HB], DBF)
            xcc = cpool.tile([128, CC1], DBF)
            cba = cpool.tile([128, 384], DBF)
            s23c = cpool.tile([64, 256], DBF)
            cbb = cpool.tile([32, ON0 + 1], DBF)
            wz = wpool.tile([128, BL], DBF)

            nc.scalar.dma_start(xca[:], xca_d[:])
            nc.gpsimd.memset(wz[:], 0.0)
            nc.gpsimd.dma_start(xcb[:], xcb_d[:])
            lnbias = wpool.tile([2 * N, 1], F32)
            nc.vector.memset(lnbias[:], LNB)
            nc.sync.dma_start(xcc[:], xcc_d[:])
            nc.sync.dma_start(cba[:], cba_d[:])
            nc.sync.dma_start(s23c[:], s23c_d[:])
            nc.sync.dma_start(cbb[:], cbb_d[:])

            wh = xca[:, WH0:WH1]
            xh0 = xca[:, XH0:XH1]
            xh1 = xcb[:]
            wl = xcc[:, WL0:WL1]
            xl = xcc[:, XL0:XL1]
            vts = cba[:, VT0:VT1]            # [128, 8*32] V^T chunks (C lhsT)
            s3 = cba[32:64, S30:S31]         # [32, 128] E3 selection (base 32)
            s2 = cbb[:, S20:S21]             # [32, 6*128] E2 selection blocks
            ones = cbb[:, ON0 : ON0 + 1]     # [32, 1]

            # ---- PE warmup in the gating PSUM bank (overwritten later) ----
            gps = psA.tile([96, BL], F32, tag="ps")
            for _ in range(N_WARMUP):
                nc.tensor.matmul(gps[:, :], wz[:, 0:96], wz[:],
                                 start=True, stop=True)

            # ---- gating: error-compensated bf16 matmul, column-split so
            # each xh half starts as soon as its own DMA lands.  Only the
            # first matmul carries start=True (start clears the whole PSUM
            # bank); the rest rely on per-element has_written semantics (the
            # warmups leave exact zeros).  The full-width Wh.xl pass runs
            # last and carries the stop for every column. ----
            nc.tensor.matmul(gps[:, 0:HB], wh, xh0, start=True, stop=False)
            nc.tensor.matmul(gps[:, HB:BL], wh, xh1, start=False, stop=False)
            nc.tensor.matmul(gps[:, 0:HB], wl, xh0, start=False, stop=False)
            nc.tensor.matmul(gps[:, HB:BL], wl, xh1, start=False, stop=False)
            nc.tensor.matmul(gps[:], wh, xl, start=False, stop=True)

            # ---- one sigmoid for all 96 gates (bf16 is plenty for 2e-2) ----
            g123 = wpool.tile([3 * N, BL], DBF)
            nc.scalar.activation(g123[:], gps[:], sig, scale=BETA)
            g2t = g123[0:32, :]
            g3t = g123[32:64, :]  # base partition 32, matching s3
            g1t = g123[64:96, :]

            # ---- E3 = S3.T @ g3t -> one DVE copy to SBUF (a TT may read at
            # most one PSUM operand); broadcast AP feeds the pair TTs ----
            e3ps = psA.tile([128, BL], F32, tag="ps")
            nc.tensor.matmul(e3ps[:], s3, g3t, start=True, stop=True)
            e3s = wpool.tile([128, BL], F32)
            nc.vector.tensor_copy(e3s[:], e3ps[:])

            # ---- ACT path for blocks 6/7: L = ln(sigma + eps) (table
            # switch hides here), matmuls sum L2[j]+L3[k], exp -> sigma2*sigma3
            lns = wpool.tile([2 * N, BL], DBF)
            nc.scalar.activation(lns[:], g123[0:64, :], fln, bias=lnbias[:])

            # ---- E2 selection matmuls for the three DVE pairs (pair 2
            # reuses pair 0's PSUM slot once pair 0's TT has drained it) ----
            e2p0 = psB.tile([128, 2, BL], F32, tag="e2")
            e2p1 = psB.tile([128, 2, BL], F32, tag="e2")
            for h in range(2):
                nc.tensor.matmul(
                    e2p0[:, h, :], s2[:, 128 * h : 128 * (h + 1)], g2t,
                    start=True, stop=True,
                )
            for h in range(2):
                q = 2 + h
                nc.tensor.matmul(
                    e2p1[:, h, :], s2[:, 128 * q : 128 * (q + 1)], g2t,
                    start=True, stop=True,
                )

            # log-sum matmuls for blocks 6/7
            lps = []
            for m in range(2):
                lp = psP.tile([128, BL], F32, tag="p23", name=f"lp_{m}")
                nc.tensor.matmul(
                    lp[:], s23c[:, 128 * m : 128 * (m + 1)], lns[:],
                    start=True, stop=True,
                )
                lps.append(lp)

            e2p2 = psB.tile([128, 2, BL], F32, tag="e2")
            for h in range(2):
                q = 4 + h
                nc.tensor.matmul(
                    e2p2[:, h, :], s2[:, 128 * q : 128 * (q + 1)], g2t,
                    start=True, stop=True,
                )

            # ---- the plane blocks ----
            at0 = apool.tile([128, 2, BL], DBF)
            at1 = apool.tile([128, 2, BL], DBF)
            at2 = apool.tile([128, 2, BL], DBF)
            e3b = e3s[:].unsqueeze(1).broadcast_to((128, 2, BL))
            nc.vector.tensor_tensor(at0[:], e2p0[:], e3b, AluOpType.mult)
            nc.vector.tensor_tensor(at1[:], e2p1[:], e3b, AluOpType.mult)
            ab = []
            for m in range(2):
                a = apool.tile([128, BL], DBF, name=f"ab_{m}")
                nc.scalar.activation(a[:], lps[m][:], fexp)
                ab.append(a)
            # pair 2 split in half so the C chain sees its blocks sooner
            nc.vector.tensor_tensor(at2[:, 0, :], e2p2[:, 0, :], e3s[:],
                                    AluOpType.mult)
            nc.vector.tensor_tensor(at2[:, 1, :], e2p2[:, 1, :], e3s[:],
                                    AluOpType.mult)

            # ---- C accumulation, ordered by expected block arrival ----
            cps = psA.tile([N, BL], F32, tag="ps")
            blocks = {0: at0[:, 0, :], 1: at0[:, 1, :],
                      2: at1[:, 0, :], 3: at1[:, 1, :],
                      4: at2[:, 0, :], 5: at2[:, 1, :],
                      6: ab[0][:], 7: ab[1][:]}
            order = [0, 1, 2, 3, 6, 7, 4, 5]
            for n_, q in enumerate(order):
                nc.tensor.matmul(
                    cps[:], vts[:, 32 * q : 32 * (q + 1)], blocks[q],
                    start=(n_ == 0), stop=(n_ == NQ - 1),
                )

            # ---- out = ones.T @ (g1t .* C^T); one DVE copy out of PSUM ----
            y = wpool.tile([N, BL], DBF)
            nc.vector.tensor_tensor(y[:], cps[:], g1t[:], AluOpType.mult)
            ops = psA.tile([1, BL], F32, tag="ps")
            nc.tensor.matmul(ops[:], ones, y[:], start=True, stop=True)
            outs = wpool.tile([1, BL], F32)
            nc.vector.tensor_copy(outs[:], ops[:])
            nc.sync.dma_start(out_d[:], outs[:], single_packet=True)

    nc.finalize()
    return nc


def host_prep(x, W1, W2, W3, V):
    """Build per-core input maps (all numpy, fp32 in / packed layouts out)."""
    x = np.asarray(x, dtype=np.float32)
    W1 = np.asarray(W1, dtype=np.float32)
    W2 = np.asarray(W2, dtype=np.float32)
    W3 = np.asarray(W3, dtype=np.float32)
    V = np.asarray(V, dtype=np.float32)

    xT = np.ascontiguousarray(x.T)  # [128, 4096]

    # order: g2 rows first (E2-mm rhs at base partition 0), then g3 (base 32,
    # matching the S3 placement), then g1 (only needed at the very end)
    Wall = np.concatenate([W2, W3, W1], axis=0)  # [96, 128]
    cf = np.ascontiguousarray(Wall.T)  # [128, 96] fp32

    # V^T chunks: VTs[p, 32q + i] = V[0, i, j, k] with jk = 128q + p
    Vr = V.reshape(N, N * N)  # [i, jk]
    VT = np.ascontiguousarray(Vr.T)  # [jk, i]
    VTs = VT.reshape(NQ, 128, N).transpose(1, 0, 2).reshape(128, NQ * N)

    # E2 selection (blocks 0-5): S2[j', q*128 + p] = 1 iff j' == 4q + p//32
    S2 = np.zeros((N, 6, 128), dtype=np.float32)
    for q in range(6):
        for p in range(128):
            S2[4 * q + p // 32, q, p] = 1.0
    S2pack = S2.reshape(N, 6 * 128)

    # E3 selection: S3[k', p] = 1 iff k' == p % 32
    S3 = np.zeros((N, 128), dtype=np.float32)
    for p in range(128):
        S3[p % 32, p] = 1.0

    # summed-log selection for blocks 6/7: one +1 weight at the L2 row
    # (j = 4q + p//32) and one at the L3 row (32 + (p % 32))
    S23 = np.zeros((2 * N, 2, 128), dtype=np.float32)
    for m, q in enumerate((6, 7)):
        for p in range(128):
            S23[4 * q + p // 32, m, p] = 1.0
            S23[N + p % 32, m, p] = 1.0
    S23pack = S23.reshape(2 * N, 2 * 128)

    cba = np.zeros((128, 384), dtype=BF16)
    cba[:, VT0:VT1] = VTs.astype(BF16)
    cba[32:64, S30:S31] = S3.astype(BF16)

    cbb = np.zeros((32, ON0 + 1), dtype=BF16)
    cbb[:, S20:S21] = S2pack.astype(BF16)
    cbb[:, ON0] = np.ones(N, dtype=BF16)

    wh = cf.astype(BF16)
    wl = (cf - wh.astype(np.float32)).astype(BF16)

    xca = np.zeros((128, CA1), dtype=BF16)
    xca[:, WH0:WH1] = wh
    xcc0 = np.zeros((128, CC1), dtype=BF16)
    xcc0[:, WL0:WL1] = wl

    in_maps = []
    for c in range(NCORES):
        ma = xca.copy()
        mc = xcc0.copy()
        xs = xT[:, c * BL : (c + 1) * BL]
        xhc = xs.astype(BF16)
        xlc = (xs - xhc.astype(np.float32)).astype(BF16)
        ma[:, XH0:XH1] = xhc[:, 0:HB]
        mc[:, XL0:XL1] = xlc
        in_maps.append(
            {"xca": ma, "xcb": np.ascontiguousarray(xhc[:, HB:BL]),
             "xcc": mc, "cba": cba, "s23c": S23pack.astype(BF16),
             "cbb": cbb}
        )
    return in_maps


_CACHED_NC = None


def _ensure_ntff_hook():
    """The agent image's `antenv` package lacks `axon_hooks`; synthesize it
    and register the boot module's ctypes-based NTFF profile hook so
    run_bass_kernel_spmd(trace=True) can capture neuron-profile output."""
    import sys, types

    try:
        from antenv.axon_hooks import get_axon_ntff_profile_hook  # noqa: F401

        return
    except ImportError:
        pass
    import antenv
    from trn_agent_boot.trn_boot import _ntff_profile_via_ctypes

    mod = types.ModuleType("antenv.axon_hooks")
    mod._hook = _ntff_profile_via_ctypes("/opt/axon/libaxon_pjrt.so")
    mod.get_axon_ntff_profile_hook = lambda: mod._hook
    mod.set_axon_ntff_profile_hook = lambda h: setattr(mod, "_hook", h)
    sys.modules["antenv.axon_hooks"] = mod
    antenv.axon_hooks = mod


def run(inputs, trace=False, **trace_kwargs):
    """Run the kernel on 8 cores. Returns (out [4096] f32, BassKernelResults)."""
    global _CACHED_NC
    if trace:
        _ensure_ntff_hook()
    if _CACHED_NC is None:
        _CACHED_NC = build_nc()
    in_maps = host_prep(
        inputs["x"], inputs["W1"], inputs["W2"], inputs["W3"], inputs["V"]
    )
    res = run_bass_kernel_spmd(
        _CACHED_NC, in_maps, core_ids=list(range(NCORES)), trace=trace, **trace_kwargs
    )
    out = np.concatenate(
        [np.asarray(res.results[c]["out"]).reshape(BL) for c in range(NCORES)]
    ).astype(np.float32)
    return out, res


def kernel(**inputs):
    out, _ = run(inputs, trace=False)
    return out
